# revision 12
# baseline (speedup 1.0000x reference)
"""Trainium2 Bass kernel for nn_CrossAttentionBlock (LN -> MHA -> out-proj -> residual).

Sharding: 8 cores = 2 batches x 4 head-groups (2 heads each). Each core:
  - LN stats via 4-way column-tiled ones-matmuls (x & x^2 for two token blocks
    concurrently); rsqrt via batched Ln,Ln,Exp,Exp (2 ACT table loads total),
  - Q/K/V projections as fp8 DoubleRow matmuls over raw x (per-token LN scale
    folded in at PSUM evacuation, mean removal as an in-group rank-1),
  - V transposed via the DMA crossbar (keeps the PE matmul stream dense),
  - attention: bf16 QK + bf16 AV with the [1|0..|V] sumexp ride-along,
  - normalization via reciprocal_approx_fast + gpsimd broadcast,
  - partial out-proj with its Wo rows, bf16 output.
Host sums the 4 partials per batch and adds bias + residual.
The PE matmul stream is kept gap-free (HAM clock-gate: the PE runs at 1.2GHz
until a ~3.4us fully-busy window promotes it to 2.4GHz, and any window with
substantial idle demotes it again). bf16 AV keeps attention PE duty high
enough to hold the fast clock; late q-projections fill the ramp.
"""
import numpy as np

C = 512
SEQ = 2048
P = 128
NB = 512         # token column block for stats/proj
DH = 64
HPC = 2          # heads per core
IG = 1024        # i-block (query) width for attention
EPS = 1e-5

_CACHE = {}
_LAST_IN_MAPS = None


def _build():
    import concourse.bass as bass
    import concourse.tile as tile
    from concourse import bacc, mybir

    F32 = mybir.dt.float32
    BF16 = mybir.dt.bfloat16
    F8 = mybir.dt.float8e4
    AF = mybir.ActivationFunctionType
    ALU = mybir.AluOpType
    DR = mybir.MatmulPerfMode.DoubleRow

    nc = bacc.Bacc("TRN2", target_bir_lowering=False, debug=False,
                   enable_asserts=False, num_devices=8)

    x8_d = nc.dram_tensor("x8", [P, 2, 2, SEQ], F8, kind="ExternalInput").ap()
    aq_d = nc.dram_tensor("aq", [P, 2, 2, P], F8, kind="ExternalInput").ap()
    ak_d = nc.dram_tensor("ak", [P, 2, 2, P], F8, kind="ExternalInput").ap()
    av_d = nc.dram_tensor("av", [P, 2, 2, P], F8, kind="ExternalInput").ap()
    wo_d = nc.dram_tensor("wo", [P, C], BF16, kind="ExternalInput").ap()
    uq_d = nc.dram_tensor("uq", [1, P], BF16, kind="ExternalInput").ap()
    uk_d = nc.dram_tensor("uk", [1, P], BF16, kind="ExternalInput").ap()
    uv_d = nc.dram_tensor("uv", [1, P], BF16, kind="ExternalInput").ap()
    vq_d = nc.dram_tensor("vq", [P, 1], F32, kind="ExternalInput").ap()
    yp_d = nc.dram_tensor("yp", [C, SEQ], BF16, kind="ExternalOutput").ap()

    with tile.TileContext(nc) as tc:
        with tc.tile_pool(name="sb", bufs=1) as sb, \
             tc.tile_pool(name="ep", bufs=4) as ep, \
             tc.tile_pool(name="pa", bufs=1, space="PSUM") as pa, \
             tc.tile_pool(name="pb", bufs=1, space="PSUM") as pb:

            # ---- x first (critical path), then weights ----
            x_f8 = sb.tile([P, 2, 2, SEQ], F8, tag="x8")
            for half in range(2):
                hs = slice(half * IG, (half + 1) * IG)
                nc.sync.dma_start(x_f8[:, :, :, hs], x8_d[:, :, :, hs])
            aw = {}
            for name, d in (("aq", aq_d), ("ak", ak_d), ("av", av_d)):
                t = sb.tile([P, 2, 2, P], F8, tag=name, name=name)
                nc.sync.dma_start(t[:], d[:, :, :, :])
                aw[name] = t
            wo_t = sb.tile([P, C], BF16, tag="wo")
            nc.sync.dma_start(wo_t[:], wo_d[:, :])
            uvec = {}
            for name, d in (("uq", uq_d), ("uk", uk_d), ("uv", uv_d)):
                t = sb.tile([1, P], BF16, tag=name, name=name)
                nc.sync.dma_start(t[:], d[:, :])
                uvec[name] = t
            vq_t = sb.tile([P, 1], F32, tag="vq")
            nc.sync.dma_start(vq_t[:], vq_d[:, :])
            ones_t = sb.tile([P, 2], F8, tag="ones")
            nc.vector.memset(ones_t[:], 1.0)
            eps_t = sb.tile([1, 1], F32, tag="eps")
            nc.vector.memset(eps_t[:], EPS)

            xsq = sb.tile([P, 2, 2, SEQ], F8, tag="xq")
            for half in range(2):
                hs = slice(half * IG, (half + 1) * IG)
                for cp in range(2):
                    nc.vector.tensor_tensor(xsq[:, cp, :, hs],
                                            x_f8[:, cp, :, hs],
                                            x_f8[:, cp, :, hs], ALU.mult)

            # ---- LN stats: 4-way column-tiled (x, x^2) x (two blocks) ----
            mu_sb = sb.tile([1, SEQ], F32, tag="mu")
            m_bf = sb.tile([1, SEQ], BF16, tag="mbf")
            e2_sb = sb.tile([1, SEQ], F32, tag="e2")
            musq = sb.tile([1, SEQ], F32, tag="musq")
            varr = sb.tile([1, SEQ], F32, tag="varr")
            lnv = sb.tile([1, SEQ], F32, tag="lnv")
            rs_row = sb.tile([1, SEQ], F32, tag="rsr")
            rs_b = sb.tile([P, SEQ], F32, tag="rsb")
            rs_bf = sb.tile([P, SEQ], BF16, tag="rsbf")
            for npair in range(2):
                nbA, nbB = 2 * npair, 2 * npair + 1
                slA = slice(nbA * NB, (nbA + 1) * NB)
                slB = slice(nbB * NB, (nbB + 1) * NB)
                st = pa.tile([P, NB], F32, tag="s0", bufs=2, name=f"st{npair}")
                groups = [(x_f8, slA), (xsq, slA), (x_f8, slB), (xsq, slB)]
                for k in range(4):
                    cp, sub = divmod(k, 2)
                    for g, (src, sl_) in enumerate(groups):
                        nc.tensor.matmul(st[32 * g:32 * g + 2, :],
                                         ones_t[:], src[:, cp, sub, sl_],
                                         start=(k == 0), stop=(k == 3),
                                         tile_position=(0, 32 * g))
                for sl_, gx in ((slA, 0), (slB, 64)):
                    nc.vector.tensor_scalar(
                        out=mu_sb[:, sl_], in0=st[gx:gx + 1, :],
                        scalar1=1.0 / C, scalar2=None, op0=ALU.mult)
                    nc.vector.tensor_scalar(
                        out=m_bf[:, sl_], in0=st[gx:gx + 1, :],
                        scalar1=1.0 / C, scalar2=None, op0=ALU.mult)
                    nc.vector.tensor_scalar(
                        out=e2_sb[:, sl_], in0=st[gx + 32:gx + 33, :],
                        scalar1=1.0 / C, scalar2=None, op0=ALU.mult)
                    nc.vector.tensor_tensor(musq[:, sl_], mu_sb[:, sl_],
                                            mu_sb[:, sl_], ALU.mult)
                    nc.vector.tensor_tensor(varr[:, sl_], e2_sb[:, sl_],
                                            musq[:, sl_], ALU.subtract)
                nc.scalar.activation(lnv[:, npair * IG:(npair + 1) * IG],
                                     varr[:, npair * IG:(npair + 1) * IG],
                                     AF.Ln, bias=eps_t[0:1, :], scale=1.0)
            for npair in range(2):
                hs = slice(npair * IG, (npair + 1) * IG)
                nc.scalar.activation(rs_row[:, hs], lnv[:, hs], AF.Exp,
                                     bias=0.0, scale=-0.5)
                nc.gpsimd.partition_broadcast(rs_b[:, hs], rs_row[:, hs],
                                              channels=P)
                nc.vector.tensor_copy(rs_bf[:, hs], rs_b[:, hs])

            # ---- projections (fp8 DoubleRow); LN folded in at evacuation ----
            # q = rs * (Aq^T x + uq (x) mu) + vq   (gamma folded into Aq)
            qt_sb = sb.tile([P, SEQ], BF16, tag="qt")
            kt_sb = sb.tile([P, SEQ], BF16, tag="kt")
            vt_sb = sb.tile([P, SEQ], BF16, tag="vt")
            # V^T via DMA crossbar transpose, then packed as [1|0*63|V64]
            v_sb = sb.tile([P, 16, 256], BF16, tag="vsb")
            nc.vector.memset(v_sb[:], 0.0)
            nc.vector.memset(
                v_sb[:].rearrange("p j (h c) -> p j h c", c=128)
                [:, :, :, 0:1], 1.0)
            vT = sb.tile([P, 16, P], BF16, tag="vT")

            def vtrans(jb):
                nc.sync.dma_start_transpose(vT[:, jb, :],
                                            vt_sb[:, jb * P:(jb + 1) * P])
                nc.vector.tensor_copy(
                    v_sb[:, jb, :].rearrange("p (h c) -> p h c", c=128)
                    [:, :, 64:128],
                    vT[:, jb, :].rearrange("p (h c) -> p h c", c=64))

            # pair i of project() calls shares one 2-bank PSUM tile
            PAIR_TAGS = ["b01", "b23", "s0", "s0", "s0", "s0"]
            pstate = {"pn": 0, "big": None}

            def project(wname, uname, dst, bias, nb):
                sl = slice(nb * NB, (nb + 1) * NB)
                pn = pstate["pn"]
                if pn % 2 == 0:
                    tag = PAIR_TAGS[pn // 2]
                    pool = pa if tag == "s0" else pb
                    pstate["big"] = pool.tile([P, 2, NB], F32, tag=tag,
                                              bufs=(2 if tag == "s0" else 1),
                                              name=f"pj{pn}")
                slot = pstate["big"][:, pn % 2, :]
                pstate["pn"] = pn + 1
                for cp in range(2):
                    nc.tensor.matmul(slot, aw[wname][:, cp, :, :],
                                     x_f8[:, cp, :, sl],
                                     start=(cp == 0), stop=False, perf_mode=DR)
                nc.tensor.matmul(slot, uvec[uname][:],
                                 m_bf[:, sl], start=False, stop=True)
                nc.vector.tensor_tensor(dst[:, sl], slot,
                                        rs_bf[:, sl], ALU.mult)
                if bias is not None:
                    nc.vector.tensor_scalar(
                        out=dst[:, sl], in0=dst[:, sl],
                        scalar1=bias[:, 0:1], scalar2=None, op0=ALU.add)
                if dst is vt_sb:
                    for jb in range(4 * nb, 4 * nb + 4):
                        vtrans(jb)

            project("ak", "uk", kt_sb, None, 0)
            project("av", "uv", vt_sb, None, 0)
            project("aq", "uq", qt_sb, vq_t, 0)
            project("ak", "uk", kt_sb, None, 1)
            project("av", "uv", vt_sb, None, 1)
            project("aq", "uq", qt_sb, vq_t, 1)
            project("ak", "uk", kt_sb, None, 2)
            project("av", "uv", vt_sb, None, 2)
            project("ak", "uk", kt_sb, None, 3)
            project("av", "uv", vt_sb, None, 3)
            # q2/q3 are deferred into the attention ramp

            # ---- attention ----
            attn_sb = sb.tile([P, SEQ], BF16, tag="at")
            yp_sb = [sb.tile([P, SEQ], BF16, tag=f"yp{m}", name=f"yp{m}")
                     for m in range(4)]
            opn = 0
            av_ps = [None, None]

            def emit_av(jb, ets):
                for h in range(HPC):
                    for nb in range(2):
                        nc.tensor.matmul(
                            av_ps[h][:, nb * 512:(nb + 1) * 512],
                            v_sb[:, jb, 128 * h:128 * h + 128],
                            ets[h][:, nb * 512:(nb + 1) * 512],
                            start=(jb == 0), stop=(jb == 15))

            def attention(ig):
                i0 = ig * IG
                prev = None
                for jb in range(16):
                    sts = []
                    for h in range(HPC):
                        sts.append(pa.tile([P, IG], F32, tag="s0", bufs=2,
                                           name=f"st{ig}_{jb}_{h}"))
                    if prev is not None:
                        emit_av(jb - 1, prev)
                    for nb in range(2):
                        for h in range(HPC):
                            hsl = slice(h * DH, (h + 1) * DH)
                            nc.tensor.matmul(
                                sts[h][:, nb * 512:(nb + 1) * 512],
                                kt_sb[hsl, jb * P:(jb + 1) * P],
                                qt_sb[hsl,
                                      i0 + nb * 512:i0 + (nb + 1) * 512],
                                start=True, stop=True,
                                tile_position=(h * DH, 0))
                    if ig == 0 and jb == 0:
                        # deferred q-proj fills the attention ramp
                        project("aq", "uq", qt_sb, vq_t, 2)
                        project("aq", "uq", qt_sb, vq_t, 3)
                    ets = []
                    for h in range(HPC):
                        e_t = ep.tile([P, IG], BF16, tag="e",
                                      name=f"e{ig}_{jb}_{h}")
                        nc.scalar.activation(e_t[:], sts[h][:],
                                             AF.Exp, bias=0.0, scale=1.0)
                        ets.append(e_t)
                    prev = ets
                emit_av(15, prev)

            def normalize_h(ig, h):
                i0 = ig * IG
                rec = sb.tile([1, IG], F32, tag=f"rc{h}", name=f"rc{ig}{h}")
                nc.vector.reciprocal_approx_fast(rec[:], av_ps[h][0:1, :])
                rb = sb.tile([P, IG], F32, tag=f"rb{h}", name=f"rb{ig}{h}")
                nc.gpsimd.partition_broadcast(rb[:], rec[:], channels=P)
                nc.vector.tensor_tensor(
                    attn_sb[h * DH:(h + 1) * DH, i0:i0 + IG],
                    av_ps[h][64:128, :], rb[64:128, :], ALU.mult)

            def outproj_m(ig, m):
                nonlocal opn
                i0 = ig * IG
                tag = "b01" if opn % 2 == 0 else "b23"
                opn += 1
                slot = pb.tile([P, IG], F32, tag=tag, name=f"op{ig}{m}")
                for nb in range(2):
                    nc.tensor.matmul(
                        slot[:, nb * 512:(nb + 1) * 512],
                        wo_t[:, m * P:(m + 1) * P],
                        attn_sb[:, i0 + nb * 512:i0 + (nb + 1) * 512],
                        start=True, stop=True)
                nc.vector.tensor_copy(yp_sb[m][:, i0:i0 + IG], slot[:])

            av_ps[0] = pb.tile([P, IG], F32, tag="b01", name="av0g0")
            av_ps[1] = pb.tile([P, IG], F32, tag="b23", name="av1g0")
            attention(0)
            normalize_h(0, 0)
            normalize_h(0, 1)
            # out-proj of ig0 (first half) fills the ig1 attention ramp
            outproj_m(0, 0)
            outproj_m(0, 1)
            av_ps[0] = pb.tile([P, IG], F32, tag="b01", name="av0g1")
            av_ps[1] = pb.tile([P, IG], F32, tag="b23", name="av1g1")
            attention(1)
            normalize_h(1, 0)
            outproj_m(0, 2)
            outproj_m(0, 3)
            normalize_h(1, 1)
            for m in range(4):
                outproj_m(1, m)
                nc.sync.dma_start(yp_d[m * P:(m + 1) * P, :], yp_sb[m][:])

    nc.compile()
    return nc


def kernel(x, Wq, Wk, Wv, Wo, bo, gamma, beta):
    import ml_dtypes
    from concourse import bass_utils

    BF = ml_dtypes.bfloat16
    F8 = ml_dtypes.float8_e4m3
    x = np.asarray(x, np.float32)
    Wq, Wk, Wv, Wo = (np.asarray(w, np.float32) for w in (Wq, Wk, Wv, Wo))
    bo, gamma, beta = (np.asarray(v, np.float32) for v in (bo, gamma, beta))
    b = x.shape[0]
    xs = x.reshape(b, C, SEQ)
    # [C, SEQ] -> fp8 DoubleRow slabs [p, cpair, sub, i]
    x8 = xs.reshape(b, 2, 2, P, SEQ).transpose(0, 3, 1, 2, 4).astype(F8)

    s = DH ** -0.5
    aq_f = gamma[:, None] * Wq * s
    ak_f = gamma[:, None] * Wk
    av_f = gamma[:, None] * Wv
    vq_f = (Wq.T @ beta) * s
    vv_f = Wv.T @ beta

    if "nc" not in _CACHE:
        _CACHE["nc"] = _build()
    nc = _CACHE["nc"]

    def wslab(w):
        # [C, 128] -> [p, cpair, sub, m] fp8
        return np.ascontiguousarray(
            w.reshape(2, 2, P, P).transpose(2, 0, 1, 3).astype(F8))

    in_maps = []
    for core in range(8):
        bi, hg = divmod(core, 4)
        cs = slice(hg * P, (hg + 1) * P)
        in_maps.append({
            "x8": np.ascontiguousarray(x8[bi]),
            "aq": wslab(aq_f[:, cs]),
            "ak": wslab(ak_f[:, cs]),
            "av": wslab(av_f[:, cs]),
            "wo": np.ascontiguousarray(Wo[cs, :].astype(BF)),
            "uq": -aq_f[:, cs].sum(0)[None, :].astype(BF),
            "uk": -ak_f[:, cs].sum(0)[None, :].astype(BF),
            "uv": -av_f[:, cs].sum(0)[None, :].astype(BF),
            "vq": vq_f[cs, None].astype(np.float32),
        })

    global _LAST_IN_MAPS
    _LAST_IN_MAPS = in_maps
    res = bass_utils.run_bass_kernel_spmd(nc, in_maps, core_ids=list(range(8)))
    bias_total = bo + Wo.T @ vv_f
    y = np.empty((b, C, SEQ), np.float32)
    for bi in range(b):
        acc = xs[bi] + bias_total[:, None]
        for hg in range(4):
            acc = acc + res.results[bi * 4 + hg]["yp"].astype(np.float32)
        y[bi] = acc
    return y.reshape(x.shape).astype(np.float32)


# revision 13
# speedup vs baseline: 1.1682x; 1.1682x over previous
"""Trainium2 Bass kernel for nn_CrossAttentionBlock (LN -> MHA -> out-proj -> residual).

Sharding: 8 cores = 2 batches x 4 head-groups (2 heads each). Each core:
  - LN stats via 4-way column-tiled ones-matmuls (x & x^2 for two token blocks
    concurrently); rsqrt via batched Ln,Ln,Exp,Exp (2 ACT table loads total),
  - Q/K/V projections as fp8 DoubleRow matmuls over raw x (per-token LN scale
    folded in at PSUM evacuation, mean removal as an in-group rank-1),
  - V transposed via the DMA crossbar (keeps the PE matmul stream dense),
  - attention: bf16 QK + bf16 AV with the [1|0..|V] sumexp ride-along,
  - normalization via reciprocal_approx_fast + gpsimd broadcast,
  - partial out-proj with its Wo rows, bf16 output.
Host sums the 4 partials per batch and adds bias + residual.
The PE matmul stream is kept gap-free (HAM clock-gate: the PE runs at 1.2GHz
until a ~3.4us fully-busy window promotes it to 2.4GHz, and any window with
substantial idle demotes it again). bf16 AV keeps attention PE duty high
enough to hold the fast clock; late q-projections fill the ramp.
"""
import numpy as np

C = 512
SEQ = 2048
P = 128
NB = 512         # token column block for stats/proj
DH = 64
HPC = 2          # heads per core
IG = 1024        # i-block (query) width for attention
EPS = 1e-5

_CACHE = {}
_LAST_IN_MAPS = None


def _build():
    import concourse.bass as bass
    import concourse.tile as tile
    from concourse import bacc, mybir

    F32 = mybir.dt.float32
    BF16 = mybir.dt.bfloat16
    F8 = mybir.dt.float8e4
    AF = mybir.ActivationFunctionType
    ALU = mybir.AluOpType
    DR = mybir.MatmulPerfMode.DoubleRow

    nc = bacc.Bacc("TRN2", target_bir_lowering=False, debug=False,
                   enable_asserts=False, num_devices=8)

    x8_d = nc.dram_tensor("x8", [P, 2, 2, SEQ], F8, kind="ExternalInput").ap()
    aq_d = nc.dram_tensor("aq", [P, 2, 2, P], F8, kind="ExternalInput").ap()
    ak_d = nc.dram_tensor("ak", [P, 2, 2, P], F8, kind="ExternalInput").ap()
    av_d = nc.dram_tensor("av", [P, 2, 2, P], F8, kind="ExternalInput").ap()
    wo_d = nc.dram_tensor("wo", [P, C], BF16, kind="ExternalInput").ap()
    uq_d = nc.dram_tensor("uq", [1, P], BF16, kind="ExternalInput").ap()
    uk_d = nc.dram_tensor("uk", [1, P], BF16, kind="ExternalInput").ap()
    uv_d = nc.dram_tensor("uv", [1, P], BF16, kind="ExternalInput").ap()
    vq_d = nc.dram_tensor("vq", [P, 1], F32, kind="ExternalInput").ap()
    yp_d = nc.dram_tensor("yp", [C, SEQ], BF16, kind="ExternalOutput").ap()

    with tile.TileContext(nc) as tc:
        with tc.tile_pool(name="sb", bufs=1) as sb, \
             tc.tile_pool(name="ep", bufs=4) as ep, \
             tc.tile_pool(name="pa", bufs=1, space="PSUM") as pa, \
             tc.tile_pool(name="pb", bufs=1, space="PSUM") as pb:

            # ---- x first (critical path), then weights ----
            x_f8 = sb.tile([P, 2, 2, SEQ], F8, tag="x8")
            for half in range(2):
                hs = slice(half * IG, (half + 1) * IG)
                nc.sync.dma_start(x_f8[:, :, :, hs], x8_d[:, :, :, hs])
            aw = {}
            for name, d in (("aq", aq_d), ("ak", ak_d), ("av", av_d)):
                t = sb.tile([P, 2, 2, P], F8, tag=name, name=name)
                nc.sync.dma_start(t[:], d[:, :, :, :])
                aw[name] = t
            wo_t = sb.tile([P, C], BF16, tag="wo")
            nc.sync.dma_start(wo_t[:], wo_d[:, :])
            uvec = {}
            for name, d in (("uq", uq_d), ("uk", uk_d), ("uv", uv_d)):
                t = sb.tile([1, P], BF16, tag=name, name=name)
                nc.sync.dma_start(t[:], d[:, :])
                uvec[name] = t
            vq_t = sb.tile([P, 1], F32, tag="vq")
            nc.sync.dma_start(vq_t[:], vq_d[:, :])
            ones_t = sb.tile([P, 2], F8, tag="ones")
            nc.vector.memset(ones_t[:], 1.0)
            eps_t = sb.tile([1, 1], F32, tag="eps")
            nc.vector.memset(eps_t[:], EPS)
            from concourse.masks import make_identity
            ident_f = sb.tile([P, P], F32, tag="idf")
            make_identity(nc, ident_f[:])
            ident_b = sb.tile([P, P], BF16, tag="idb")
            nc.vector.tensor_copy(ident_b[:], ident_f[:])

            xsq = sb.tile([P, 2, 2, SEQ], F8, tag="xq")
            for half in range(2):
                hs = slice(half * IG, (half + 1) * IG)
                for cp in range(2):
                    nc.vector.tensor_tensor(xsq[:, cp, :, hs],
                                            x_f8[:, cp, :, hs],
                                            x_f8[:, cp, :, hs], ALU.mult)

            # ---- LN stats: 4-way column-tiled (x, x^2) x (two blocks) ----
            mu_sb = sb.tile([1, SEQ], F32, tag="mu")
            m_bf = sb.tile([1, SEQ], BF16, tag="mbf")
            e2_sb = sb.tile([1, SEQ], F32, tag="e2")
            musq = sb.tile([1, SEQ], F32, tag="musq")
            varr = sb.tile([1, SEQ], F32, tag="varr")
            lnv = sb.tile([1, SEQ], F32, tag="lnv")
            rs_row = sb.tile([1, SEQ], F32, tag="rsr")
            rs_b = sb.tile([P, SEQ], F32, tag="rsb")
            rs_bf = sb.tile([P, SEQ], BF16, tag="rsbf")
            for npair in range(2):
                nbA, nbB = 2 * npair, 2 * npair + 1
                slA = slice(nbA * NB, (nbA + 1) * NB)
                slB = slice(nbB * NB, (nbB + 1) * NB)
                st = pa.tile([P, NB], F32, tag="s0", bufs=2, name=f"st{npair}")
                groups = [(x_f8, slA), (xsq, slA), (x_f8, slB), (xsq, slB)]
                for k in range(4):
                    cp, sub = divmod(k, 2)
                    for g, (src, sl_) in enumerate(groups):
                        nc.tensor.matmul(st[32 * g:32 * g + 2, :],
                                         ones_t[:], src[:, cp, sub, sl_],
                                         start=(k == 0), stop=(k == 3),
                                         tile_position=(0, 32 * g))
                for sl_, gx in ((slA, 0), (slB, 64)):
                    nc.vector.tensor_scalar(
                        out=mu_sb[:, sl_], in0=st[gx:gx + 1, :],
                        scalar1=1.0 / C, scalar2=None, op0=ALU.mult)
                    nc.vector.tensor_scalar(
                        out=m_bf[:, sl_], in0=st[gx:gx + 1, :],
                        scalar1=1.0 / C, scalar2=None, op0=ALU.mult)
                    nc.vector.tensor_scalar(
                        out=e2_sb[:, sl_], in0=st[gx + 32:gx + 33, :],
                        scalar1=1.0 / C, scalar2=None, op0=ALU.mult)
                    nc.vector.tensor_tensor(musq[:, sl_], mu_sb[:, sl_],
                                            mu_sb[:, sl_], ALU.mult)
                    nc.vector.tensor_tensor(varr[:, sl_], e2_sb[:, sl_],
                                            musq[:, sl_], ALU.subtract)
                nc.scalar.activation(lnv[:, npair * IG:(npair + 1) * IG],
                                     varr[:, npair * IG:(npair + 1) * IG],
                                     AF.Ln, bias=eps_t[0:1, :], scale=1.0)
            for npair in range(2):
                hs = slice(npair * IG, (npair + 1) * IG)
                nc.scalar.activation(rs_row[:, hs], lnv[:, hs], AF.Exp,
                                     bias=0.0, scale=-0.5)
                nc.gpsimd.partition_broadcast(rs_b[:, hs], rs_row[:, hs],
                                              channels=P)
                nc.vector.tensor_copy(rs_bf[:, hs], rs_b[:, hs])

            # ---- projections (fp8 DoubleRow); LN folded in at evacuation ----
            # q = rs * (Aq^T x + uq (x) mu) + vq   (gamma folded into Aq)
            qt_sb = sb.tile([P, SEQ], BF16, tag="qt")
            kt_sb = sb.tile([P, SEQ], BF16, tag="kt")
            vt_sb = sb.tile([P, SEQ], BF16, tag="vt")
            # V^T via DMA crossbar transpose, then packed as [1|0*63|V64]
            v_sb = sb.tile([P, 16, 256], BF16, tag="vsb")
            nc.vector.memset(v_sb[:], 0.0)
            nc.vector.memset(
                v_sb[:].rearrange("p j (h c) -> p j h c", c=128)
                [:, :, :, 0:1], 1.0)
            def vtrans(jb):
                tr = pa.tile([P, P], BF16, tag="s0", bufs=2, name=f"tr{jb}")
                nc.tensor.transpose(tr[:], vt_sb[:, jb * P:(jb + 1) * P],
                                    ident_b[:])
                nc.vector.tensor_copy(
                    v_sb[:, jb, :].rearrange("p (h c) -> p h c", c=128)
                    [:, :, 64:128],
                    tr[:].rearrange("p (h c) -> p h c", c=64))

            # pair i of project() calls shares one 2-bank PSUM tile
            PAIR_TAGS = ["b01", "b23", "s0", "s0", "s0", "s0"]
            # order: all K/V pairs, then V transposes, then a dense q-pair
            # burst as the HAM warm-entry right before attention
            pstate = {"pn": 0, "big": None}

            def project(wname, uname, dst, bias, nb):
                sl = slice(nb * NB, (nb + 1) * NB)
                pn = pstate["pn"]
                if pn % 2 == 0:
                    tag = PAIR_TAGS[pn // 2]
                    pool = pa if tag == "s0" else pb
                    pstate["big"] = pool.tile([P, 2, NB], F32, tag=tag,
                                              bufs=(2 if tag == "s0" else 1),
                                              name=f"pj{pn}")
                slot = pstate["big"][:, pn % 2, :]
                pstate["pn"] = pn + 1
                for cp in range(2):
                    nc.tensor.matmul(slot, aw[wname][:, cp, :, :],
                                     x_f8[:, cp, :, sl],
                                     start=(cp == 0), stop=False, perf_mode=DR)
                nc.tensor.matmul(slot, uvec[uname][:],
                                 m_bf[:, sl], start=False, stop=True)
                nc.vector.tensor_tensor(dst[:, sl], slot,
                                        rs_bf[:, sl], ALU.mult)
                if bias is not None:
                    nc.vector.tensor_scalar(
                        out=dst[:, sl], in0=dst[:, sl],
                        scalar1=bias[:, 0:1], scalar2=None, op0=ALU.add)
            project("ak", "uk", kt_sb, None, 0)
            project("av", "uv", vt_sb, None, 0)
            project("ak", "uk", kt_sb, None, 1)
            project("av", "uv", vt_sb, None, 1)
            project("ak", "uk", kt_sb, None, 2)
            project("av", "uv", vt_sb, None, 2)
            project("ak", "uk", kt_sb, None, 3)
            project("av", "uv", vt_sb, None, 3)
            for jb in range(16):
                vtrans(jb)
            project("aq", "uq", qt_sb, vq_t, 0)
            project("aq", "uq", qt_sb, vq_t, 1)
            project("aq", "uq", qt_sb, vq_t, 2)
            project("aq", "uq", qt_sb, vq_t, 3)

            # ---- attention ----
            attn_sb = sb.tile([P, SEQ], BF16, tag="at")
            yp_sb = [sb.tile([P, SEQ], BF16, tag=f"yp{m}", name=f"yp{m}")
                     for m in range(4)]
            opn = 0
            av_ps = [None, None]

            def emit_av(jb, ets):
                for h in range(HPC):
                    for nb in range(2):
                        nc.tensor.matmul(
                            av_ps[h][:, nb * 512:(nb + 1) * 512],
                            v_sb[:, jb, 128 * h:128 * h + 128],
                            ets[h][:, nb * 512:(nb + 1) * 512],
                            start=(jb == 0), stop=(jb == 15))

            def attention(ig):
                i0 = ig * IG
                prev = None
                for jb in range(16):
                    sts = []
                    for h in range(HPC):
                        sts.append(pa.tile([P, IG], F32, tag="s0", bufs=2,
                                           name=f"st{ig}_{jb}_{h}"))
                    if prev is not None:
                        emit_av(jb - 1, prev)
                    for nb in range(2):
                        for h in range(HPC):
                            hsl = slice(h * DH, (h + 1) * DH)
                            nc.tensor.matmul(
                                sts[h][:, nb * 512:(nb + 1) * 512],
                                kt_sb[hsl, jb * P:(jb + 1) * P],
                                qt_sb[hsl,
                                      i0 + nb * 512:i0 + (nb + 1) * 512],
                                start=True, stop=True,
                                tile_position=(h * DH, 0))
                    ets = []
                    for h in range(HPC):
                        e_t = ep.tile([P, IG], BF16, tag="e",
                                      name=f"e{ig}_{jb}_{h}")
                        nc.scalar.activation(e_t[:], sts[h][:],
                                             AF.Exp, bias=0.0, scale=1.0)
                        ets.append(e_t)
                    prev = ets
                emit_av(15, prev)

            def normalize_h(ig, h):
                i0 = ig * IG
                rec = sb.tile([1, IG], F32, tag=f"rc{h}", name=f"rc{ig}{h}")
                nc.vector.reciprocal_approx_fast(rec[:], av_ps[h][0:1, :])
                rb = sb.tile([P, IG], F32, tag=f"rb{h}", name=f"rb{ig}{h}")
                nc.gpsimd.partition_broadcast(rb[:], rec[:], channels=P)
                nc.vector.tensor_tensor(
                    attn_sb[h * DH:(h + 1) * DH, i0:i0 + IG],
                    av_ps[h][64:128, :], rb[64:128, :], ALU.mult)

            def outproj_m(ig, m):
                nonlocal opn
                i0 = ig * IG
                tag = "b01" if opn % 2 == 0 else "b23"
                opn += 1
                slot = pb.tile([P, IG], F32, tag=tag, name=f"op{ig}{m}")
                for nb in range(2):
                    nc.tensor.matmul(
                        slot[:, nb * 512:(nb + 1) * 512],
                        wo_t[:, m * P:(m + 1) * P],
                        attn_sb[:, i0 + nb * 512:i0 + (nb + 1) * 512],
                        start=True, stop=True)
                nc.vector.tensor_copy(yp_sb[m][:, i0:i0 + IG], slot[:])

            av_ps[0] = pb.tile([P, IG], F32, tag="b01", name="av0g0")
            av_ps[1] = pb.tile([P, IG], F32, tag="b23", name="av1g0")
            attention(0)
            normalize_h(0, 0)
            normalize_h(0, 1)
            # out-proj of ig0 (first half) fills the ig1 attention ramp
            outproj_m(0, 0)
            outproj_m(0, 1)
            av_ps[0] = pb.tile([P, IG], F32, tag="b01", name="av0g1")
            av_ps[1] = pb.tile([P, IG], F32, tag="b23", name="av1g1")
            attention(1)
            normalize_h(1, 0)
            outproj_m(0, 2)
            outproj_m(0, 3)
            normalize_h(1, 1)
            for m in range(4):
                outproj_m(1, m)
                nc.sync.dma_start(yp_d[m * P:(m + 1) * P, :], yp_sb[m][:])

    nc.compile()
    return nc


def kernel(x, Wq, Wk, Wv, Wo, bo, gamma, beta):
    import ml_dtypes
    from concourse import bass_utils

    BF = ml_dtypes.bfloat16
    F8 = ml_dtypes.float8_e4m3
    x = np.asarray(x, np.float32)
    Wq, Wk, Wv, Wo = (np.asarray(w, np.float32) for w in (Wq, Wk, Wv, Wo))
    bo, gamma, beta = (np.asarray(v, np.float32) for v in (bo, gamma, beta))
    b = x.shape[0]
    xs = x.reshape(b, C, SEQ)
    # [C, SEQ] -> fp8 DoubleRow slabs [p, cpair, sub, i]
    x8 = xs.reshape(b, 2, 2, P, SEQ).transpose(0, 3, 1, 2, 4).astype(F8)

    s = DH ** -0.5
    aq_f = gamma[:, None] * Wq * s
    ak_f = gamma[:, None] * Wk
    av_f = gamma[:, None] * Wv
    vq_f = (Wq.T @ beta) * s
    vv_f = Wv.T @ beta

    if "nc" not in _CACHE:
        _CACHE["nc"] = _build()
    nc = _CACHE["nc"]

    def wslab(w):
        # [C, 128] -> [p, cpair, sub, m] fp8
        return np.ascontiguousarray(
            w.reshape(2, 2, P, P).transpose(2, 0, 1, 3).astype(F8))

    in_maps = []
    for core in range(8):
        bi, hg = divmod(core, 4)
        cs = slice(hg * P, (hg + 1) * P)
        in_maps.append({
            "x8": np.ascontiguousarray(x8[bi]),
            "aq": wslab(aq_f[:, cs]),
            "ak": wslab(ak_f[:, cs]),
            "av": wslab(av_f[:, cs]),
            "wo": np.ascontiguousarray(Wo[cs, :].astype(BF)),
            "uq": -aq_f[:, cs].sum(0)[None, :].astype(BF),
            "uk": -ak_f[:, cs].sum(0)[None, :].astype(BF),
            "uv": -av_f[:, cs].sum(0)[None, :].astype(BF),
            "vq": vq_f[cs, None].astype(np.float32),
        })

    global _LAST_IN_MAPS
    _LAST_IN_MAPS = in_maps
    res = bass_utils.run_bass_kernel_spmd(nc, in_maps, core_ids=list(range(8)))
    bias_total = bo + Wo.T @ vv_f
    y = np.empty((b, C, SEQ), np.float32)
    for bi in range(b):
        acc = xs[bi] + bias_total[:, None]
        for hg in range(4):
            acc = acc + res.results[bi * 4 + hg]["yp"].astype(np.float32)
        y[bi] = acc
    return y.reshape(x.shape).astype(np.float32)
